# revision 22
# baseline (speedup 1.0000x reference)
"""Trainium2 Bass kernel for nn_NeuralECMModel (GAT-style segment softmax + scatter).

Math (from the reference):
    nodes are all-zero  =>  s_tgt = 0
    per edge value x:   p = w*x ;  s = p*a_src ;  e = leaky_relu(s, 0.2) ; ex = exp(e)
    per node (segment): d = sum(ex) ; u = sum(p*ex)
    out = elu(u/(d+1e-16) + bias) @ rank_W.T + rank_b

For the canonical inputs, segment_ids == repeat(arange(N), 51) (each node owns a
contiguous run of exactly 51 edges) and edge_feats values are exactly {0.0, 1.0}.
Both properties are verified on the host; when they hold, ex is linear in x:
    ex = 1 + x*(ex1-1)   with  ex1 = exp(leaky_relu(w*a_src))
so only S_n = sum(x) per segment is needed on-device:
    out_n = elu( (w*ex1*S_n) / ((ex1-1)*S_n + 51 + 1e-16) + bias ) * rW + rb

Device-side strategy (memory-regime):
  - Host re-encodes edge_feats as uint8 (exact: values are {0,1}) with each
    51-edge segment padded to 52 bytes -> 3.25 MB/core instead of 12.75 MB.
  - Big chunks are laid out as two 26-byte half-planes; an SWDGE DMA with
    accum_op=add folds plane B into plane A during the transfer, so the DVE
    tree starts from 26 bytes/segment.
  - Grouped sums use a SWAR tree: pairs of bytes viewed as uint16 lanes
    (lane sums <= 26 < 256, never carry) with contiguous-half tensor_tensor
    adds in the DVE 2x_1p mode (0.52 ns/elem).
  - Per-segment S feeds: den = A*S+B ; r ~= 1/den (fast reciprocal);
    z = ALPHA*r + BETA  (folds q = S/den = 1/A - (B/A) r);
    out = RW*min(exp(z)-1, relu(z)) + RB.
  - Emission is software-pipelined (input DMAs all issue first, trees run
    as chunks land, pointwise stages are deferred one chunk apiece) and the
    output ships one DMA per group so only a tiny transfer trails the tail.
If any host-side property fails, an exact numpy fallback replicates the
reference bit-for-bit semantics.
"""

import numpy as np

N_NODES = 500_000
DEG1 = 51
E = N_NODES * DEG1
N_CORES = 8
SEGS_PER_CORE = N_NODES // N_CORES       # 62500 segments per core
P = 125                                  # SBUF partitions used
SEGS_PER_PART = SEGS_PER_CORE // P       # 500 segments per partition
SEG_B = 52                               # padded bytes per segment (26 u16)
HALF_B = SEG_B // 2                      # accum-DMA half-plane bytes
POOL_SEG_B = 56                          # pool-engine chunks: 14 u32 per seg


def _layout():
    """Per-chunk byte layout: (sizes, byte-offsets, row bytes)."""
    sizes = [c for c, _ in CHUNKS]
    bs = [c * (POOL_SEG_B if m == "pool" else SEG_B) for c, m in CHUNKS]
    boffs = [0]
    for b in bs:
        boffs.append(boffs[-1] + b)
    return sizes, boffs, boffs[-1]


ROW_B = SEGS_PER_PART * SEG_B            # updated after CHUNKS is final
ROW_F = ROW_B                            # bytes per partition (for test.py)

# (segments, mode) per chunk; must sum to SEGS_PER_PART.  'accum' chunks are
# laid out as two half-planes and folded by an accumulate-DMA; 'plain' chunks
# ramp the pipeline up/down (SWDGE desc-gen is ~1us, too slow for small ones).
CHUNKS = [(16, "plain"), (78, "accum"), (34, "plain"), (78, "accum"),
          (78, "accum"), (78, "accum"), (78, "accum"), (48, "accum"),
          (12, "plain")]
# pointwise runs on batched groups of chunks (fewer fixed-cost ops); each
# entry is the index one past the last chunk of the group
GROUPS = [4, 7, 9]
# input-DMA issue order (indices into CHUNKS); accum plane-A transfers early
# start the SWDGE desc-gen chain sooner
DMA_ORDER = None

_CACHE = {}
LAST_RESULTS = None  # BassKernelResults of the most recent device run


def _leaky(v):
    return v if v >= 0.0 else np.float32(0.2) * v


def _fallback(query_emb, entity_emb, edge_feats, segment_ids, W_proj, a_src,
              a_tgt, bias, rank_W, rank_b):
    """Exact numpy replica of the reference for non-canonical inputs."""
    n = entity_emb.shape[0]
    x = edge_feats.astype(np.float32)
    proj_e = x @ W_proj.T.astype(np.float32)                  # [E,1]
    s_src = (proj_e * a_src.astype(np.float32)).sum(-1)       # [E]
    nodes = np.zeros((n, 1), np.float32)
    proj_n = nodes @ W_proj.T.astype(np.float32)
    s_tgt = (proj_n * a_tgt.astype(np.float32)).sum(-1)       # [n] (zeros)
    e = (s_src + s_tgt[segment_ids]).astype(np.float32)
    e = np.where(e >= 0, e, np.float32(0.2) * e).astype(np.float32)
    ex = np.exp(e).astype(np.float32)
    denom = np.bincount(segment_ids, weights=ex.astype(np.float64),
                        minlength=n).astype(np.float32)
    attn = (ex / (denom[segment_ids] + np.float32(1e-16))).astype(np.float32)
    num = np.bincount(segment_ids,
                      weights=(proj_e[:, 0] * attn).astype(np.float64),
                      minlength=n).astype(np.float32)
    z = (num[:, None] + bias.astype(np.float32)).astype(np.float32)
    y = np.where(z > 0, z, np.expm1(z)).astype(np.float32)
    return (y @ rank_W.T.astype(np.float32) + rank_b.astype(np.float32)
            ).astype(np.float32)


def _build(consts):
    """Build + schedule the Tile program for one core (SPMD across 8)."""
    import concourse.bacc as bacc
    import concourse.tile as tile
    from concourse import mybir

    A, B, ALPHA, BETA, RW, RB = consts

    nc = bacc.Bacc("TRN2", target_bir_lowering=False,
                   debug=False, num_devices=N_CORES)
    x_d = nc.dram_tensor("x", [P, _layout()[2]], mybir.dt.uint8,
                         kind="ExternalInput").ap()
    o_d = nc.dram_tensor("o", [P, SEGS_PER_PART], mybir.dt.float32,
                         kind="ExternalOutput").ap()

    f32 = mybir.dt.float32
    u8 = mybir.dt.uint8
    u16 = mybir.dt.uint16
    u32 = mybir.dt.uint32
    AF = mybir.ActivationFunctionType
    ALU = mybir.AluOpType

    K = len(CHUNKS)
    sizes, boffs, row_b = _layout()
    assert sum(sizes) == SEGS_PER_PART
    offs = np.concatenate([[0], np.cumsum(sizes)]).tolist()
    cmax = max(sizes)
    gmax = max(offs[e] - (offs[GROUPS[i - 1]] if i else 0)
               for i, e in enumerate(GROUPS))
    # o = RW*elu(z) + RB collapses into 3 ops when RB==0 and RW>0:
    #   min(exp(z + ln RW) - RW, relu(RW*z))
    collapse = (RB == 0.0 and RW > 0.0)

    def flat(ap):
        return ap.rearrange("p c e -> p (c e)")

    with tile.TileContext(nc) as tc:
        with tc.tile_pool(name="xs", bufs=3) as xs, \
             tc.tile_pool(name="singles", bufs=1) as singles, \
             tc.tile_pool(name="mid", bufs=2) as mid, \
             tc.tile_pool(name="small", bufs=3) as small:
            b_den = singles.tile([P, 1], f32)
            nc.vector.memset(b_den, float(B))
            if collapse:
                b_e = singles.tile([P, 1], f32)
                nc.vector.memset(b_e, float(BETA) + float(np.log(RW)))
                b_r = singles.tile([P, 1], f32)
                nc.vector.memset(b_r, float(RW) * float(BETA))
            else:
                b_z = singles.tile([P, 1], f32)
                nc.vector.memset(b_z, float(BETA))
                b_rb = singles.tile([P, 1], f32)
                nc.vector.memset(b_rb, float(RB))
            out_t = singles.tile([P, SEGS_PER_PART], f32)
            tbuf = singles.tile([P, SEGS_PER_PART], u16)

            tiles = [None] * K

            def stage_dma(k):
                """Input DMA for chunk k (SP hwdge; accum: plane A only)."""
                c, mode = CHUNKS[k]
                off_b = boffs[k]
                nb = (c * HALF_B if mode == "accum"
                      else c * POOL_SEG_B if mode == "pool" else c * SEG_B)
                xt = xs.tile([P, nb], u8, tag=f"x{k}", name="xt", bufs=1)
                nc.sync.dma_start(out=xt, in_=x_d[:, off_b:off_b + nb])
                tiles[k] = xt

            def stage_tree(k):
                """SWAR tree for chunk k into tbuf (engine per mode)."""
                c, mode = CHUNKS[k]
                off_b = boffs[k]
                if mode == "pool":
                    # whole tree in u32 lanes on the Pool engine (integer
                    # adds are u32/i32-only there); byte lanes <= 14 each
                    v32 = tiles[k].bitcast(u32).rearrange(
                        "p (c e) -> p c e", e=14)
                    t7 = mid.tile([P, cmax, 7], u32, tag="t7",
                                  name="t7")[:, :c]
                    nc.gpsimd.tensor_tensor(out=t7, in0=v32[:, :, 0:7],
                                            in1=v32[:, :, 7:14], op=ALU.add)
                    t3p = mid.tile([P, cmax, 3], u32, tag="t3p",
                                   name="t3p")[:, :c]
                    nc.gpsimd.tensor_tensor(out=t3p, in0=t7[:, :, 0:3],
                                            in1=t7[:, :, 3:6], op=ALU.add)
                    uap = small.tile([P, cmax], u32, tag="uap",
                                     name="uap")[:, :c]
                    nc.gpsimd.tensor_tensor(out=uap, in0=flat(t3p[:, :, 0:1]),
                                            in1=flat(t3p[:, :, 1:2]),
                                            op=ALU.add)
                    ubp = small.tile([P, cmax], u32, tag="ubp",
                                     name="ubp")[:, :c]
                    nc.gpsimd.tensor_tensor(out=ubp, in0=flat(t3p[:, :, 2:3]),
                                            in1=flat(t7[:, :, 6:7]),
                                            op=ALU.add)
                    t32 = small.tile([P, cmax], u32, tag="t32",
                                     name="t32")[:, :c]
                    nc.gpsimd.tensor_tensor(out=t32, in0=uap, in1=ubp,
                                            op=ALU.add)
                    # fold the two u16 halves (lanes add: lo+hi u16) on DVE
                    t16 = t32.bitcast(u16).rearrange("p (c e) -> p c e", e=2)
                    nc.vector.tensor_tensor(out=tbuf[:, offs[k]:offs[k] + c],
                                            in0=flat(t16[:, :, 0:1]),
                                            in1=flat(t16[:, :, 1:2]),
                                            op=ALU.add)
                    return
                if mode == "plain":
                    xt_c = tiles[k]
                    v = xt_c.bitcast(u16).rearrange("p (c e) -> p c e", e=26)
                    t13 = mid.tile([P, cmax, 13], u16, tag="t13",
                                   name="t13")[:, :c]
                    nc.vector.tensor_tensor(out=t13, in0=v[:, :, 0:13],
                                            in1=v[:, :, 13:26], op=ALU.add)
                else:
                    xa_c = tiles[k]
                    # SWDGE accumulate corrupts contiguous runs > 2048B
                    # (tail descriptor lands misaligned), so accum chunks
                    # keep c*HALF_B <= 2028 (verified exact on HW)
                    assert c * HALF_B <= 2028
                    nc.gpsimd.dma_start(
                        out=xa_c,
                        in_=x_d[:, off_b + c * HALF_B:off_b + 2 * c * HALF_B],
                        accum_op=ALU.add)  # noqa: layout has B after A
                    t13 = xa_c.bitcast(u16).rearrange("p (c e) -> p c e", e=13)

                t6 = mid.tile([P, cmax, 6], u16, tag="t6", name="t6")[:, :c]
                nc.vector.tensor_tensor(out=t6, in0=t13[:, :, 0:6],
                                        in1=t13[:, :, 6:12], op=ALU.add)
                t3 = mid.tile([P, cmax, 3], u16, tag="t3", name="t3")[:, :c]
                nc.vector.tensor_tensor(out=t3, in0=t6[:, :, 0:3],
                                        in1=t6[:, :, 3:6], op=ALU.add)
                ua = small.tile([P, cmax], u16, tag="ua", name="ua")[:, :c]
                nc.vector.tensor_tensor(out=ua, in0=flat(t3[:, :, 0:1]),
                                        in1=flat(t3[:, :, 1:2]), op=ALU.add)
                ub = small.tile([P, cmax], u16, tag="ub", name="ub")[:, :c]
                nc.vector.tensor_tensor(out=ub, in0=flat(t3[:, :, 2:3]),
                                        in1=flat(t13[:, :, 12:13]), op=ALU.add)
                nc.vector.tensor_tensor(out=tbuf[:, offs[k]:offs[k] + c],
                                        in0=ua, in1=ub, op=ALU.add)

            gst = [dict() for _ in GROUPS]

            def grng(gi):
                g0 = offs[GROUPS[gi - 1]] if gi else 0
                return g0, offs[GROUPS[gi]]

            def gp_stage1(gi):
                """byte-split S = lo + hi (DVE) and den = A*S+B (Act)."""
                g0, g1 = grng(gi)
                c = g1 - g0
                t8 = tbuf[:, g0:g1].bitcast(u8).rearrange(
                    "p (c e) -> p c e", e=2)
                s8 = small.tile([P, gmax], u8, tag="s8", name="s8")[:, :c]
                nc.vector.tensor_tensor(out=s8, in0=flat(t8[:, :, 0:1]),
                                        in1=flat(t8[:, :, 1:2]), op=ALU.add)
                den = small.tile([P, gmax], f32, tag="den", name="den")[:, :c]
                nc.scalar.activation(den, s8, AF.Identity, bias=b_den,
                                     scale=float(A))
                gst[gi]["den"] = den

            def gp_stage2(gi):
                """r ~= 1/den (DVE) then exp/relu branches (Act)."""
                g0, g1 = grng(gi)
                c = g1 - g0
                r = small.tile([P, gmax], f32, tag="r", name="r")[:, :c]
                nc.vector.reciprocal_approx_fast(out=r, in_=gst[gi]["den"])
                el = small.tile([P, gmax], f32, tag="el", name="el")[:, :c]
                rl = small.tile([P, gmax], f32, tag="rl", name="rl")[:, :c]
                if collapse:
                    nc.scalar.activation(el, r, AF.Exp, bias=b_e,
                                         scale=float(ALPHA))
                    nc.scalar.activation(rl, r, AF.Relu, bias=b_r,
                                         scale=float(RW) * float(ALPHA))
                else:
                    nc.scalar.activation(el, r, AF.Exp, bias=b_z,
                                         scale=float(ALPHA))
                    nc.scalar.activation(rl, r, AF.Relu, bias=b_z,
                                         scale=float(ALPHA))
                gst[gi].update(el=el, rl=rl)

            def gp_stage3(gi):
                """final elu combine into the output tile."""
                g0, g1 = grng(gi)
                c = g1 - g0
                el, rl = gst[gi]["el"], gst[gi]["rl"]
                out_sl = out_t[:, g0:g1]
                if collapse:
                    nc.vector.scalar_tensor_tensor(
                        out=out_sl, in0=el, scalar=-float(RW), in1=rl,
                        op0=ALU.add, op1=ALU.min)
                else:
                    y = small.tile([P, gmax], f32, tag="y", name="y")[:, :c]
                    nc.vector.scalar_tensor_tensor(out=y, in0=el, scalar=-1.0,
                                                   in1=rl, op0=ALU.add,
                                                   op1=ALU.min)
                    nc.scalar.activation(out_sl, y, AF.Identity, bias=b_rb,
                                         scale=float(RW))

            # all plane-A/plain input DMAs first: the SP queue streams them
            # back-to-back while SWDGE desc-gen (~1us each, serialized on
            # Pool) trickles the accumulate transfers in behind
            for k in (DMA_ORDER or range(K)):
                stage_dma(k)
            # trees in chunk order; group pointwise stages are deferred one
            # chunk apiece so no engine stalls on a cross-engine round-trip
            defer = []
            gi = 0
            for k in range(K):
                stage_tree(k)
                if defer:
                    defer.pop(0)()
                if gi < len(GROUPS) and k + 1 == GROUPS[gi]:
                    g = gi
                    defer.append(lambda g=g: gp_stage1(g))
                    defer.append(lambda g=g: gp_stage2(g))
                    defer.append(lambda g=g: gp_stage3(g))
                    gi += 1
                    defer.pop(0)()
            while defer:
                defer.pop(0)()
            # one output DMA per group: early groups ship while later ones
            # still compute; only the last (tiny) transfer trails the tail
            for gi2 in range(len(GROUPS)):
                g0, g1 = grng(gi2)
                nc.sync.dma_start(out=o_d[:, g0:g1], in_=out_t[:, g0:g1])

    nc.compile()
    return nc


def _get_nc(consts):
    key = tuple(float(v) for v in consts)
    if key not in _CACHE:
        _CACHE[key] = _build(consts)
    return _CACHE[key]


def _pack_u8(x):
    """[E,1] f32 {0,1} -> per-core uint8 [N_CORES, P, row_b].

    'plain' chunks: segments padded to 52B, seg-major.
    'accum' chunks: two 26B half-planes (plane A = seg bytes 0:26,
    plane B = seg bytes 26:51 + pad), each seg-major within its plane.
    'pool' chunks: segments padded to 56B (14 u32 lanes), seg-major.
    """
    _, boffs, row_b = _layout()
    xv = x.reshape(N_CORES, P, SEGS_PER_PART, DEG1)
    out = np.empty((N_CORES, P, row_b), np.uint8)
    off = 0
    for k, (c, mode) in enumerate(CHUNKS):
        sb = POOL_SEG_B if mode == "pool" else SEG_B
        seg = np.zeros((N_CORES, P, c, sb), np.uint8)
        seg[..., :DEG1] = xv[:, :, off:off + c]
        dst = out[:, :, boffs[k]:boffs[k + 1]]
        if mode == "accum":
            dst[:, :, :c * HALF_B] = seg[..., :HALF_B].reshape(
                N_CORES, P, c * HALF_B)
            dst[:, :, c * HALF_B:] = seg[..., HALF_B:].reshape(
                N_CORES, P, c * HALF_B)
        else:
            dst[:] = seg.reshape(N_CORES, P, c * sb)
        off += c
    return out


def kernel(**inputs):
    x = np.ascontiguousarray(inputs["edge_feats"])
    seg = inputs["segment_ids"]
    W_proj = inputs["W_proj"]
    a_src = inputs["a_src"]
    bias = inputs["bias"]
    rank_W = inputs["rank_W"]
    rank_b = inputs["rank_b"]

    fast = (x.shape == (E, 1) and seg.shape == (E,)
            and inputs["entity_emb"].shape[0] == N_NODES)
    if fast:
        seg2 = seg.reshape(N_NODES, DEG1)
        fast = bool((seg2[:, 0] == np.arange(N_NODES, dtype=seg.dtype)).all()
                    and (seg2 == seg2[:, :1]).all())
    if fast:
        xf = x.reshape(-1)
        fast = bool(((xf == np.float32(0.0)) | (xf == np.float32(1.0))).all())

    # host-side scalar folding (f32 chain to mirror the reference)
    w = np.float32(W_proj.reshape(-1)[0])
    a = np.float32(a_src.reshape(-1)[0])
    c = np.float32(w * a)
    k = _leaky(c)
    ex1 = np.float32(np.exp(np.float32(k)))
    A = np.float32(ex1 - np.float32(1.0))       # den = A*S + B
    B = np.float32(np.float32(DEG1) + np.float32(1e-16))
    SC = np.float32(w * ex1)                    # z = SC*(S/den) + bias
    if fast:
        # q = S/den = 1/A - (B/A)*r needs A bounded away from 0
        fast = bool(abs(float(A)) > 1e-5)
    if not fast:
        return _fallback(**inputs)

    BIAS = np.float32(bias.reshape(-1)[0])
    RW = np.float32(rank_W.reshape(-1)[0])
    RB = np.float32(rank_b.reshape(-1)[0])
    ALPHA = np.float32(-float(SC) * float(B) / float(A))
    BETA = np.float32(float(SC) / float(A) + float(BIAS))

    from concourse import bass_utils
    nc = _get_nc((A, B, ALPHA, BETA, RW, RB))

    xp = _pack_u8(x)
    in_maps = [{"x": xp[i]} for i in range(N_CORES)]
    res = bass_utils.run_bass_kernel_spmd(nc, in_maps,
                                          core_ids=list(range(N_CORES)))
    global LAST_RESULTS
    LAST_RESULTS = res
    out = np.concatenate([r["o"].reshape(-1) for r in res.results])
    return out.reshape(N_NODES, 1).astype(np.float32)
